# revision 2
# baseline (speedup 1.0000x reference)
"""BitLinear Trainium2 kernel: out = x @ (unpack_bits(bp) * scale).T

Full-input contract: kernel(x, bp, scale) -> [8192, 16384] float32.

Strategy (column-parallel tensor parallelism across 8 NeuronCores):
- Shard bp/scale along out_features (2048 per core); replicate x.
- Weights are exactly representable in bf16 (values are +/-1), so split
  fp32 x into bf16 hi + bf16 lo parts and accumulate both matmuls into
  the same fp32 PSUM group: fp32-grade accuracy at bf16 PE throughput.
- Host pre-transposes x to [in, batch] and pre-decodes the bit matrix to
  B.T [in, out_shard] bf16 so the device loop is pure DMA + matmul.
- Scale is applied during PSUM->SBUF eviction on VectorE.
"""

import os

import numpy as np
import ml_dtypes

BATCH = 8192
IN_FEATURES = 4096
OUT_FEATURES = 16384
N_CORES = 8
O_PER_CORE = OUT_FEATURES // N_CORES  # 2048

P = 128
N_FREE = 512  # moving free dim / PSUM bank (fp32)
K_TILES = IN_FEATURES // P  # 32
B_TILES = BATCH // P  # 64
O_TILES = O_PER_CORE // N_FREE  # 4

_CACHE = {}


def _split_multi_waits(nc, mybir, bass_rust):
    """The walrus build here supports one sem-wait per instruction; Tile's
    final drain aggregates several. Move excess waits onto preceding nops."""
    for f in nc.m.functions:
        for b in f.blocks:
            new_insts = []
            for inst in b.instructions:
                si = inst.sync_info
                if si and si.on_wait and len(si.on_wait) > 1:
                    waits = list(si.on_wait)
                    for j, w in enumerate(waits[:-1]):
                        nop = mybir.InstNoOp(
                            name=f"{inst.name}-waitsplit-{j}", ins=[], outs=[]
                        )
                        nop.engine = inst.engine
                        nop.sync_info = bass_rust.SyncInfo(on_wait=[w], on_update=[])
                        new_insts.append(nop)
                    inst.sync_info = bass_rust.SyncInfo(
                        on_wait=[waits[-1]], on_update=list(si.on_update)
                    )
                new_insts.append(inst)
            b.instructions[:] = new_insts


def _build():
    import concourse.bass as bass
    import concourse.mybir as mybir
    import bass_rust
    from concourse.tile import TileContext

    dt = mybir.dt
    nc = bass.Bass()

    xhi = nc.dram_tensor("xhi", (IN_FEATURES, BATCH), dt.bfloat16, kind="ExternalInput")
    xlo = nc.dram_tensor("xlo", (IN_FEATURES, BATCH), dt.bfloat16, kind="ExternalInput")
    bt = nc.dram_tensor("bt", (IN_FEATURES, O_PER_CORE), dt.bfloat16, kind="ExternalInput")
    scale = nc.dram_tensor("scale", (P, O_PER_CORE), dt.float32, kind="ExternalInput")
    out = nc.dram_tensor("out", (BATCH, O_PER_CORE), dt.float32, kind="ExternalOutput")

    bt_r = bt.rearrange("(k p) o -> p k o", p=P)  # [128, 32, 2048]
    xhi_r = xhi.rearrange("(k p) b -> p k b", p=P)  # [128, 32, 8192]
    xlo_r = xlo.rearrange("(k p) b -> p k b", p=P)

    with TileContext(nc) as tc:
        with (
            tc.tile_pool(name="wpool", bufs=1) as wpool,
            tc.tile_pool(name="spool", bufs=1) as spool,
            tc.tile_pool(name="xpool", bufs=3) as xpool,
            tc.tile_pool(name="opool", bufs=6) as opool,
            tc.tile_pool(name="psum", bufs=8, space="PSUM") as psum_pool,
        ):
            # Resident weights (16 MB = 128 KB/partition) + scale row block.
            wt = wpool.tile([P, K_TILES, O_PER_CORE], dt.bfloat16)
            nc.sync.dma_start(out=wt[:], in_=bt_r)
            sc = spool.tile([P, O_PER_CORE], dt.float32)
            nc.sync.dma_start(out=sc[:], in_=scale[:, :])

            for bi in range(B_TILES):
                xh = xpool.tile([P, K_TILES, P], dt.bfloat16, tag="xh")
                xl = xpool.tile([P, K_TILES, P], dt.bfloat16, tag="xl")
                nc.sync.dma_start(out=xh[:], in_=xhi_r[:, :, bass.ts(bi, P)])
                nc.sync.dma_start(out=xl[:], in_=xlo_r[:, :, bass.ts(bi, P)])

                psums = [
                    psum_pool.tile([P, N_FREE], dt.float32, tag="ps", name="ps")
                    for _ in range(O_TILES)
                ]
                for k in range(K_TILES):
                    for oi in range(O_TILES):
                        nc.tensor.matmul(
                            psums[oi][:],
                            xh[:, k, :],
                            wt[:, k, bass.ts(oi, N_FREE)],
                            start=(k == 0),
                            stop=False,
                        )
                    for oi in range(O_TILES):
                        nc.tensor.matmul(
                            psums[oi][:],
                            xl[:, k, :],
                            wt[:, k, bass.ts(oi, N_FREE)],
                            start=False,
                            stop=(k == K_TILES - 1),
                        )
                for oi in range(O_TILES):
                    ot = opool.tile([P, N_FREE], dt.float32, tag="ot")
                    nc.vector.tensor_mul(ot[:], psums[oi][:], sc[:, bass.ts(oi, N_FREE)])
                    nc.sync.dma_start(
                        out=out[bass.ts(bi, P), bass.ts(oi, N_FREE)], in_=ot[:]
                    )

    _split_multi_waits(nc, mybir, bass_rust)
    return nc


def _prep_inputs(x, bp, scale):
    bf16 = ml_dtypes.bfloat16
    x = np.asarray(x, dtype=np.float32)
    x_hi = x.astype(bf16)
    x_lo = (x - x_hi.astype(np.float32)).astype(bf16)
    xhiT = np.ascontiguousarray(x_hi.T)  # [4096, 8192] bf16
    xloT = np.ascontiguousarray(x_lo.T)

    bits = np.unpackbits(np.asarray(bp, dtype=np.uint8))  # MSB-first, matches ref
    b_mat = bits.reshape(OUT_FEATURES, IN_FEATURES).astype(np.int8)
    b_mat = (b_mat << 1) - 1  # {0,1} -> {-1,+1}

    scale = np.asarray(scale, dtype=np.float32).reshape(OUT_FEATURES)

    in_maps = []
    for c in range(N_CORES):
        sl = slice(c * O_PER_CORE, (c + 1) * O_PER_CORE)
        btT = np.ascontiguousarray(b_mat[sl].T).astype(bf16)  # [4096, 2048]
        sc_b = np.ascontiguousarray(
            np.broadcast_to(scale[sl][None, :], (P, O_PER_CORE))
        )
        in_maps.append({"xhi": xhiT, "xlo": xloT, "bt": btT, "scale": sc_b})
    return in_maps


def kernel(x, bp, scale):
    from concourse import bass_utils

    if "nc" not in _CACHE:
        _CACHE["nc"] = _build()
    nc = _CACHE["nc"]

    in_maps = _prep_inputs(x, bp, scale)

    trace = bool(os.environ.get("BITLINEAR_TRACE"))
    res = bass_utils.run_bass_kernel_spmd(
        nc, in_maps, core_ids=list(range(N_CORES)), trace=trace
    )
    _CACHE["last_exec_time_ns"] = res.exec_time_ns
    _CACHE["last_results"] = res

    out = np.concatenate([res.results[c]["out"] for c in range(N_CORES)], axis=1)
    return np.ascontiguousarray(out)


# revision 4
# speedup vs baseline: 1.9591x; 1.9591x over previous
"""BitLinear Trainium2 kernel: out = x @ (unpack_bits(bp) * scale).T

Full-input contract: kernel(x, bp, scale) -> [8192, 16384] float32.

Strategy (column-parallel tensor parallelism across 8 NeuronCores):
- Shard bp/scale along out_features (2048 per core); replicate x.
- Weights are exactly +/-1, hence exactly representable in 16-bit floats.
  The only quantization error is on x, so:
    * mode "bf16_pair": split fp32 x into bf16 hi + bf16 lo and accumulate
      both matmul passes into the same fp32 PSUM group (~2.5e-6 rel err).
    * mode "fp16": single fp16 pass (~2.8e-4 rel err, half the PE work).
- Host pre-transposes x to [in, batch] and pre-decodes the bit matrix to
  B.T [in, out_shard] so the device loop is pure DMA + matmul.
- Scale is applied during PSUM->SBUF eviction on VectorE.
"""

import os

import numpy as np
import ml_dtypes

BATCH = 8192
IN_FEATURES = 4096
OUT_FEATURES = 16384
N_CORES = 8
O_PER_CORE = OUT_FEATURES // N_CORES  # 2048

P = 128
N_FREE = 512  # moving free dim / one PSUM bank of fp32
K_TILES = IN_FEATURES // P  # 32
B_TILES = BATCH // P  # 64
O_TILES = O_PER_CORE // N_FREE  # 4

# "bf16_pair": x split into bf16 hi+lo, 2 accumulating passes (~2.5e-6 rel err)
# "fp16": single fp16 pass (~2.8e-4 rel err, 2x faster)
MODE = os.environ.get("BITLINEAR_MODE", "bf16_pair")

_CACHE = {}


def _split_multi_waits(nc, mybir, bass_rust):
    """The walrus build here supports one sem-wait per instruction; Tile's
    final drain aggregates several. Move excess waits onto preceding nops."""
    for f in nc.m.functions:
        for b in f.blocks:
            new_insts = []
            for inst in b.instructions:
                si = inst.sync_info
                if si and si.on_wait and len(si.on_wait) > 1:
                    waits = list(si.on_wait)
                    for j, w in enumerate(waits[:-1]):
                        nop = mybir.InstNoOp(
                            name=f"{inst.name}-waitsplit-{j}", ins=[], outs=[]
                        )
                        nop.engine = inst.engine
                        nop.sync_info = bass_rust.SyncInfo(on_wait=[w], on_update=[])
                        new_insts.append(nop)
                    inst.sync_info = bass_rust.SyncInfo(
                        on_wait=[waits[-1]], on_update=list(si.on_update)
                    )
                new_insts.append(inst)
            b.instructions[:] = new_insts


def _mode_config(mode):
    if mode == "bf16_pair":
        return ["xhi", "xlo"], "bfloat16"
    elif mode == "fp16":
        return ["xhi"], "float16"
    raise ValueError(f"unknown mode {mode}")


def _build(mode):
    import concourse.bass as bass
    import concourse.mybir as mybir
    import bass_rust
    from concourse.tile import TileContext

    part_names, dt_name = _mode_config(mode)
    dt = mybir.dt
    xdt = getattr(dt, dt_name)
    nc = bass.Bass()

    xparts = [
        nc.dram_tensor(nm, (IN_FEATURES, BATCH), xdt, kind="ExternalInput")
        for nm in part_names
    ]
    bt = nc.dram_tensor("bt", (IN_FEATURES, O_PER_CORE), xdt, kind="ExternalInput")
    scale = nc.dram_tensor("scale", (P, O_PER_CORE), dt.float32, kind="ExternalInput")
    out = nc.dram_tensor("out", (BATCH, O_PER_CORE), dt.float32, kind="ExternalOutput")

    bt_r = bt.rearrange("(k p) o -> p k o", p=P)  # [128, 32, 2048]
    xparts_r = [xp.rearrange("(k p) b -> p k b", p=P) for xp in xparts]  # [128,32,8192]
    n_parts = len(xparts)

    with TileContext(nc) as tc:
        with (
            tc.tile_pool(name="wpool", bufs=1) as wpool,
            tc.tile_pool(name="spool", bufs=1) as spool,
            tc.tile_pool(name="xpool", bufs=3) as xpool,
            tc.tile_pool(name="opool", bufs=6) as opool,
            tc.tile_pool(name="psum", bufs=8, space="PSUM") as psum_pool,
        ):
            # Resident weights (16 MB = 128 KB/partition) + scale row block.
            # Weight DMA split by k so early matmuls can start sooner.
            wt = wpool.tile([P, K_TILES, O_PER_CORE], xdt)
            for k in range(K_TILES):
                nc.sync.dma_start(out=wt[:, k, :], in_=bt_r[:, k, :])
            sc = spool.tile([P, O_PER_CORE], dt.float32)
            nc.sync.dma_start(out=sc[:], in_=scale[:, :])

            for bi in range(B_TILES):
                xts = []
                for pi in range(n_parts):
                    xt = xpool.tile([P, K_TILES, P], xdt, tag=f"x{pi}", name=f"x{pi}")
                    nc.sync.dma_start(out=xt[:], in_=xparts_r[pi][:, :, bass.ts(bi, P)])
                    xts.append(xt)

                psums = [
                    psum_pool.tile([P, N_FREE], dt.float32, tag="ps", name="ps")
                    for _ in range(O_TILES)
                ]
                for k in range(K_TILES):
                    for pi in range(n_parts):
                        for oi in range(O_TILES):
                            nc.tensor.matmul(
                                psums[oi][:],
                                xts[pi][:, k, :],
                                wt[:, k, bass.ts(oi, N_FREE)],
                                start=(k == 0 and pi == 0),
                                stop=(k == K_TILES - 1 and pi == n_parts - 1),
                            )
                for oi in range(O_TILES):
                    ot = opool.tile([P, N_FREE], dt.float32, tag="ot", name="ot")
                    nc.vector.tensor_mul(ot[:], psums[oi][:], sc[:, bass.ts(oi, N_FREE)])
                    nc.sync.dma_start(
                        out=out[bass.ts(bi, P), bass.ts(oi, N_FREE)], in_=ot[:]
                    )

    _split_multi_waits(nc, mybir, bass_rust)
    return nc


def _prep_inputs(x, bp, scale, mode):
    part_names, dt_name = _mode_config(mode)
    np_xdt = dict(bfloat16=ml_dtypes.bfloat16, float16=np.float16)[dt_name]

    x = np.asarray(x, dtype=np.float32)
    xT = np.ascontiguousarray(x.T)  # [4096, 8192] fp32
    parts = {}
    resid = xT
    for i, nm in enumerate(part_names):
        q = resid.astype(np_xdt)
        parts[nm] = q
        if i + 1 < len(part_names):
            resid = resid - q.astype(np.float32)

    bits = np.unpackbits(np.asarray(bp, dtype=np.uint8))  # MSB-first, matches ref
    b_mat = bits.reshape(OUT_FEATURES, IN_FEATURES).astype(np.int8)
    b_mat = (b_mat << 1) - 1  # {0,1} -> {-1,+1}

    scale = np.asarray(scale, dtype=np.float32).reshape(OUT_FEATURES)

    in_maps = []
    for c in range(N_CORES):
        sl = slice(c * O_PER_CORE, (c + 1) * O_PER_CORE)
        btT = np.ascontiguousarray(b_mat[sl].T).astype(np_xdt)  # [4096, 2048]
        sc_b = np.ascontiguousarray(
            np.broadcast_to(scale[sl][None, :], (P, O_PER_CORE))
        )
        in_maps.append({**parts, "bt": btT, "scale": sc_b})
    return in_maps


def kernel(x, bp, scale):
    from concourse import bass_utils

    key = ("nc", MODE)
    if key not in _CACHE:
        _CACHE[key] = _build(MODE)
    nc = _CACHE[key]

    in_maps = _prep_inputs(x, bp, scale, MODE)

    trace = bool(os.environ.get("BITLINEAR_TRACE"))
    res = bass_utils.run_bass_kernel_spmd(
        nc, in_maps, core_ids=list(range(N_CORES)), trace=trace
    )
    _CACHE["last_exec_time_ns"] = res.exec_time_ns
    _CACHE["last_results"] = res

    out = np.concatenate([res.results[c]["out"] for c in range(N_CORES)], axis=1)
    return np.ascontiguousarray(out)


# revision 5
# speedup vs baseline: 1.9861x; 1.0138x over previous
"""BitLinear Trainium2 kernel: out = x @ (unpack_bits(bp) * scale).T

Full-input contract: kernel(x, bp, scale) -> [8192, 16384] float32.

Strategy (column-parallel tensor parallelism across 8 NeuronCores):
- Shard bp/scale along out_features (2048 per core); replicate x.
- Weights are exactly +/-1, hence exactly representable in 16-bit floats.
  The only quantization error is on x, so:
    * mode "bf16_pair": split fp32 x into bf16 hi + bf16 lo and accumulate
      both matmul passes into the same fp32 PSUM group (~2.5e-6 rel err).
    * mode "fp16": single fp16 pass (~2.8e-4 rel err, half the PE work).
- Host pre-transposes x to [in, batch] and pre-decodes the bit matrix to
  B.T [in, out_shard] so the device loop is pure DMA + matmul.
- Scale is applied during PSUM->SBUF eviction on VectorE.
"""

import os

import numpy as np
import ml_dtypes

BATCH = 8192
IN_FEATURES = 4096
OUT_FEATURES = 16384
N_CORES = 8
O_PER_CORE = OUT_FEATURES // N_CORES  # 2048

P = 128
N_FREE = 512  # moving free dim / one PSUM bank of fp32
K_TILES = IN_FEATURES // P  # 32
B_TILES = BATCH // P  # 64
O_TILES = O_PER_CORE // N_FREE  # 4

# "bf16_pair": x split into bf16 hi+lo, 2 accumulating passes (~2.5e-6 rel err)
# "fp16": single fp16 pass (~2.8e-4 rel err, 2x faster)
MODE = os.environ.get("BITLINEAR_MODE", "bf16_pair")

_CACHE = {}


def _split_multi_waits(nc, mybir, bass_rust):
    """The walrus build here supports one sem-wait per instruction; Tile's
    final drain aggregates several. Move excess waits onto preceding nops."""
    for f in nc.m.functions:
        for b in f.blocks:
            new_insts = []
            for inst in b.instructions:
                si = inst.sync_info
                if si and si.on_wait and len(si.on_wait) > 1:
                    waits = list(si.on_wait)
                    for j, w in enumerate(waits[:-1]):
                        nop = mybir.InstNoOp(
                            name=f"{inst.name}-waitsplit-{j}", ins=[], outs=[]
                        )
                        nop.engine = inst.engine
                        nop.sync_info = bass_rust.SyncInfo(on_wait=[w], on_update=[])
                        new_insts.append(nop)
                    inst.sync_info = bass_rust.SyncInfo(
                        on_wait=[waits[-1]], on_update=list(si.on_update)
                    )
                new_insts.append(inst)
            b.instructions[:] = new_insts


def _mode_config(mode):
    if mode == "bf16_pair":
        return ["xhi", "xlo"], "bfloat16"
    elif mode == "fp16":
        return ["xhi"], "float16"
    raise ValueError(f"unknown mode {mode}")


def _build(mode):
    import concourse.bass as bass
    import concourse.mybir as mybir
    import bass_rust
    from concourse.tile import TileContext

    part_names, dt_name = _mode_config(mode)
    dt = mybir.dt
    xdt = getattr(dt, dt_name)
    nc = bass.Bass()

    xparts = [
        nc.dram_tensor(nm, (IN_FEATURES, BATCH), xdt, kind="ExternalInput")
        for nm in part_names
    ]
    bt = nc.dram_tensor("bt", (IN_FEATURES, O_PER_CORE), xdt, kind="ExternalInput")
    scale = nc.dram_tensor("scale", (P, O_PER_CORE), dt.float32, kind="ExternalInput")
    out = nc.dram_tensor("out", (BATCH, O_PER_CORE), dt.float32, kind="ExternalOutput")

    bt_r = bt.rearrange("(k p) o -> p k o", p=P)  # [128, 32, 2048]
    xparts_r = [xp.rearrange("(k p) b -> p k b", p=P) for xp in xparts]  # [128,32,8192]
    n_parts = len(xparts)

    with TileContext(nc) as tc:
        with (
            tc.tile_pool(name="wpool", bufs=1) as wpool,
            tc.tile_pool(name="spool", bufs=1) as spool,
            tc.tile_pool(name="xpool", bufs=3) as xpool,
            tc.tile_pool(name="opool", bufs=6) as opool,
            tc.tile_pool(name="psum", bufs=8, space="PSUM") as psum_pool,
        ):
            def load_x(bi):
                xts = []
                for pi in range(n_parts):
                    xt = xpool.tile([P, K_TILES, P], xdt, tag=f"x{pi}", name=f"x{pi}")
                    nc.sync.dma_start(out=xt[:], in_=xparts_r[pi][:, :, bass.ts(bi, P)])
                    xts.append(xt)
                return xts

            # First x tile before the bulk weight load so matmuls can start
            # as soon as the k=0 weight slice lands (startup was 67us when
            # the 33 weight/scale DMAs were all emitted first).
            prefetched = load_x(0)

            # Resident weights (128 KB/partition) split by k; scale last
            # (first needed only after the first full psum, ~30us in).
            wt = wpool.tile([P, K_TILES, O_PER_CORE], xdt)
            for k in range(K_TILES):
                nc.sync.dma_start(out=wt[:, k, :], in_=bt_r[:, k, :])
            sc = spool.tile([P, O_PER_CORE], dt.float32)
            nc.sync.dma_start(out=sc[:], in_=scale[:, :])

            for bi in range(B_TILES):
                xts = prefetched
                if bi + 1 < B_TILES:
                    prefetched = load_x(bi + 1)

                psums = [
                    psum_pool.tile([P, N_FREE], dt.float32, tag="ps", name="ps")
                    for _ in range(O_TILES)
                ]
                for k in range(K_TILES):
                    for pi in range(n_parts):
                        for oi in range(O_TILES):
                            nc.tensor.matmul(
                                psums[oi][:],
                                xts[pi][:, k, :],
                                wt[:, k, bass.ts(oi, N_FREE)],
                                start=(k == 0 and pi == 0),
                                stop=(k == K_TILES - 1 and pi == n_parts - 1),
                            )
                for oi in range(O_TILES):
                    ot = opool.tile([P, N_FREE], dt.float32, tag="ot", name="ot")
                    nc.vector.tensor_mul(ot[:], psums[oi][:], sc[:, bass.ts(oi, N_FREE)])
                    nc.sync.dma_start(
                        out=out[bass.ts(bi, P), bass.ts(oi, N_FREE)], in_=ot[:]
                    )

    _split_multi_waits(nc, mybir, bass_rust)
    return nc


def _prep_inputs(x, bp, scale, mode):
    part_names, dt_name = _mode_config(mode)
    np_xdt = dict(bfloat16=ml_dtypes.bfloat16, float16=np.float16)[dt_name]

    x = np.asarray(x, dtype=np.float32)
    xT = np.ascontiguousarray(x.T)  # [4096, 8192] fp32
    parts = {}
    resid = xT
    for i, nm in enumerate(part_names):
        q = resid.astype(np_xdt)
        parts[nm] = q
        if i + 1 < len(part_names):
            resid = resid - q.astype(np.float32)

    bits = np.unpackbits(np.asarray(bp, dtype=np.uint8))  # MSB-first, matches ref
    b_mat = bits.reshape(OUT_FEATURES, IN_FEATURES).astype(np.int8)
    b_mat = (b_mat << 1) - 1  # {0,1} -> {-1,+1}

    scale = np.asarray(scale, dtype=np.float32).reshape(OUT_FEATURES)

    in_maps = []
    for c in range(N_CORES):
        sl = slice(c * O_PER_CORE, (c + 1) * O_PER_CORE)
        btT = np.ascontiguousarray(b_mat[sl].T).astype(np_xdt)  # [4096, 2048]
        sc_b = np.ascontiguousarray(
            np.broadcast_to(scale[sl][None, :], (P, O_PER_CORE))
        )
        in_maps.append({**parts, "bt": btT, "scale": sc_b})
    return in_maps


def kernel(x, bp, scale):
    from concourse import bass_utils

    key = ("nc", MODE)
    if key not in _CACHE:
        _CACHE[key] = _build(MODE)
    nc = _CACHE[key]

    in_maps = _prep_inputs(x, bp, scale, MODE)

    trace = bool(os.environ.get("BITLINEAR_TRACE"))
    res = bass_utils.run_bass_kernel_spmd(
        nc, in_maps, core_ids=list(range(N_CORES)), trace=trace
    )
    _CACHE["last_exec_time_ns"] = res.exec_time_ns
    _CACHE["last_results"] = res

    out = np.concatenate([res.results[c]["out"] for c in range(N_CORES)], axis=1)
    return np.ascontiguousarray(out)


# revision 7
# speedup vs baseline: 2.0053x; 1.0097x over previous
"""BitLinear Trainium2 kernel: out = x @ (unpack_bits(bp) * scale).T

Full-input contract: kernel(x, bp, scale) -> [8192, 16384] float32.

Strategy (column-parallel tensor parallelism across 8 NeuronCores):
- Shard bp/scale along out_features (2048 per core); replicate x.
- Weights are exactly +/-1, hence exactly representable in 16-bit floats.
  The only quantization error is on x, so:
    * mode "bf16_pair": split fp32 x into bf16 hi + bf16 lo and accumulate
      both matmul passes into the same fp32 PSUM group (~2.5e-6 rel err).
    * mode "fp16": single fp16 pass (~2.8e-4 rel err, half the PE work).
- Host pre-transposes x to [in, batch] and pre-decodes the bit matrix to
  B.T [in, out_shard] so the device loop is pure DMA + matmul.
- Scale is applied during PSUM->SBUF eviction on VectorE.
"""

import os

import numpy as np
import ml_dtypes

BATCH = 8192
IN_FEATURES = 4096
OUT_FEATURES = 16384
N_CORES = 8
O_PER_CORE = OUT_FEATURES // N_CORES  # 2048

P = 128
N_FREE = 512  # moving free dim / one PSUM bank of fp32
K_TILES = IN_FEATURES // P  # 32
B_TILES = BATCH // P  # 64
O_TILES = O_PER_CORE // N_FREE  # 4

# "bf16_pair": x split into bf16 hi+lo, 2 accumulating passes (~2.5e-6 rel err)
# "fp16": single fp16 pass (~2.8e-4 rel err, 2x faster)
MODE = os.environ.get("BITLINEAR_MODE", "bf16_pair")

_CACHE = {}


def _split_multi_waits(nc, mybir, bass_rust):
    """The walrus build here supports one sem-wait per instruction; Tile's
    final drain aggregates several. Move excess waits onto preceding nops."""
    for f in nc.m.functions:
        for b in f.blocks:
            new_insts = []
            for inst in b.instructions:
                si = inst.sync_info
                if si and si.on_wait and len(si.on_wait) > 1:
                    waits = list(si.on_wait)
                    for j, w in enumerate(waits[:-1]):
                        nop = mybir.InstNoOp(
                            name=f"{inst.name}-waitsplit-{j}", ins=[], outs=[]
                        )
                        nop.engine = inst.engine
                        nop.sync_info = bass_rust.SyncInfo(on_wait=[w], on_update=[])
                        new_insts.append(nop)
                    inst.sync_info = bass_rust.SyncInfo(
                        on_wait=[waits[-1]], on_update=list(si.on_update)
                    )
                new_insts.append(inst)
            b.instructions[:] = new_insts


def _mode_config(mode):
    if mode == "bf16_pair":
        return ["xhi", "xlo"], "bfloat16"
    elif mode == "fp16":
        return ["xhi"], "float16"
    raise ValueError(f"unknown mode {mode}")


def _build(mode):
    import concourse.bass as bass
    import concourse.mybir as mybir
    import bass_rust
    from concourse.tile import TileContext

    part_names, dt_name = _mode_config(mode)
    dt = mybir.dt
    xdt = getattr(dt, dt_name)
    nc = bass.Bass()

    xparts = [
        nc.dram_tensor(nm, (IN_FEATURES, BATCH), xdt, kind="ExternalInput")
        for nm in part_names
    ]
    bt = nc.dram_tensor("bt", (IN_FEATURES, O_PER_CORE), xdt, kind="ExternalInput")
    scale = nc.dram_tensor("scale", (P, O_PER_CORE), dt.float32, kind="ExternalInput")
    out = nc.dram_tensor("out", (BATCH, O_PER_CORE), dt.float32, kind="ExternalOutput")

    bt_r = bt.rearrange("(k p) o -> p k o", p=P)  # [128, 32, 2048]
    xparts_r = [xp.rearrange("(k p) b -> p k b", p=P) for xp in xparts]  # [128,32,8192]
    n_parts = len(xparts)

    with TileContext(nc) as tc:
        with (
            tc.tile_pool(name="wpool", bufs=1) as wpool,
            tc.tile_pool(name="spool", bufs=1) as spool,
            tc.tile_pool(name="xpool", bufs=3) as xpool,
            tc.tile_pool(name="opool", bufs=6) as opool,
            tc.tile_pool(name="psum", bufs=8, space="PSUM") as psum_pool,
        ):
            def load_x(bi):
                xts = []
                for pi in range(n_parts):
                    xt = xpool.tile([P, K_TILES, P], xdt, tag=f"x{pi}", name=f"x{pi}")
                    nc.sync.dma_start(out=xt[:], in_=xparts_r[pi][:, :, bass.ts(bi, P)])
                    xts.append(xt)
                return xts

            # Warm the PE HAM clock gate (1.2 -> 2.4 GHz needs ~3.4us of
            # sustained matmul activity) with dummy matmuls on a zeroed tile
            # while the first DMAs are still in flight.
            warm = spool.tile([P, N_FREE], xdt, name="warm")
            nc.vector.memset(warm[:], 0.0)
            warm_ps = psum_pool.tile([P, N_FREE], dt.float32, tag="ps", name="warm_ps")
            for _ in range(12):
                nc.tensor.matmul(
                    warm_ps[:], warm[:, :P], warm[:], start=True, stop=True
                )

            # First two x tiles before the bulk weight load so matmuls can
            # start as soon as the k=0 weight slice lands and the bi=0 -> 1
            # transition doesn't wait on a queued-last x DMA (startup was
            # 67us when the weight/scale DMAs were all emitted first).
            prefetched = load_x(0)
            prefetched2 = load_x(1)

            # Resident weights (128 KB/partition) split by k; scale last
            # (first needed only after the first full psum, ~30us in).
            wt = wpool.tile([P, K_TILES, O_PER_CORE], xdt)
            for k in range(K_TILES):
                nc.sync.dma_start(out=wt[:, k, :], in_=bt_r[:, k, :])
            sc = spool.tile([P, O_PER_CORE], dt.float32)
            nc.sync.dma_start(out=sc[:], in_=scale[:, :])

            for bi in range(B_TILES):
                xts = prefetched
                prefetched = prefetched2
                if bi + 2 < B_TILES:
                    prefetched2 = load_x(bi + 2)

                psums = [
                    psum_pool.tile([P, N_FREE], dt.float32, tag="ps", name="ps")
                    for _ in range(O_TILES)
                ]
                for k in range(K_TILES):
                    for pi in range(n_parts):
                        for oi in range(O_TILES):
                            nc.tensor.matmul(
                                psums[oi][:],
                                xts[pi][:, k, :],
                                wt[:, k, bass.ts(oi, N_FREE)],
                                start=(k == 0 and pi == 0),
                                stop=(k == K_TILES - 1 and pi == n_parts - 1),
                            )
                for oi in range(O_TILES):
                    ot = opool.tile([P, N_FREE], dt.float32, tag="ot", name="ot")
                    nc.vector.tensor_mul(ot[:], psums[oi][:], sc[:, bass.ts(oi, N_FREE)])
                    nc.sync.dma_start(
                        out=out[bass.ts(bi, P), bass.ts(oi, N_FREE)], in_=ot[:]
                    )

    _split_multi_waits(nc, mybir, bass_rust)
    return nc


def _prep_inputs(x, bp, scale, mode):
    part_names, dt_name = _mode_config(mode)
    np_xdt = dict(bfloat16=ml_dtypes.bfloat16, float16=np.float16)[dt_name]

    x = np.asarray(x, dtype=np.float32)
    xT = np.ascontiguousarray(x.T)  # [4096, 8192] fp32
    parts = {}
    resid = xT
    for i, nm in enumerate(part_names):
        q = resid.astype(np_xdt)
        parts[nm] = q
        if i + 1 < len(part_names):
            resid = resid - q.astype(np.float32)

    bits = np.unpackbits(np.asarray(bp, dtype=np.uint8))  # MSB-first, matches ref
    b_mat = bits.reshape(OUT_FEATURES, IN_FEATURES).astype(np.int8)
    b_mat = (b_mat << 1) - 1  # {0,1} -> {-1,+1}

    scale = np.asarray(scale, dtype=np.float32).reshape(OUT_FEATURES)

    in_maps = []
    for c in range(N_CORES):
        sl = slice(c * O_PER_CORE, (c + 1) * O_PER_CORE)
        btT = np.ascontiguousarray(b_mat[sl].T).astype(np_xdt)  # [4096, 2048]
        sc_b = np.ascontiguousarray(
            np.broadcast_to(scale[sl][None, :], (P, O_PER_CORE))
        )
        in_maps.append({**parts, "bt": btT, "scale": sc_b})
    return in_maps


def kernel(x, bp, scale):
    from concourse import bass_utils

    key = ("nc", MODE)
    if key not in _CACHE:
        _CACHE[key] = _build(MODE)
    nc = _CACHE[key]

    in_maps = _prep_inputs(x, bp, scale, MODE)

    trace = bool(os.environ.get("BITLINEAR_TRACE"))
    res = bass_utils.run_bass_kernel_spmd(
        nc, in_maps, core_ids=list(range(N_CORES)), trace=trace
    )
    _CACHE["last_exec_time_ns"] = res.exec_time_ns
    _CACHE["last_results"] = res

    out = np.concatenate([res.results[c]["out"] for c in range(N_CORES)], axis=1)
    return np.ascontiguousarray(out)
